# revision 30
# baseline (speedup 1.0000x reference)
"""Self-contained Trainium2 Bass kernel for nn_Attention_xxc_52390011077379.

kernel(**inputs) takes FULL inputs:
  x [8192, 17, 512] f32, W_qkv [512, 1536], W_proj [512, 512], b_proj [512]
returns FULL output [8192, 17, 512] f32.

Data-parallel over batch across 8 NeuronCores (1024 frames/core, padded to
1036 = 148 groups of 7 frames).

Final design (HW exec ~711us vs 1524us baseline):
 - q,k: 1-term fp8e4m3 DoubleRow matmuls (2 passes of 256-contraction) from
   host-quantized xq8/w8; descale 2^-10 on the PSUM->SBUF copy.
 - v: f16 matmuls (x channel-major stationary, Wv moving), row-major out.
 - scores/AV: f16 block-diag per 7-frame group (GR=119), h-dim stored
   par-major (h' = par*4+qi) so score copies write contiguously; AV picks
   the permuted v head; host permutes W_proj rows to match.
 - b-major softmax spine with NO strided writes: (n,m,h) layout end-to-end,
   exp contiguous; sum over m via contiguous fold-tree (8+8+1); normalize
   reads strided, writes (m,h,n) contiguous.
 - bonechain batched into 5 regular-stride op-pairs (13 steps).
 - strip<->b-major partition regroups via DRAM staging: fat single-DMA
   writes (conv1a full sstrip, conv2a full bmP), 272B-run reads split
   across gpsimd/sync rings by j-parity.
 - software pipeline: loads(i+1); front(i)=A2,B,A3,conv1a; spine(i);
   finish(i-2) [3 rotating at-strips]; single shared 8-slot PSUM pool.
 - proj emits channel-major yT f16; host transposes back, adds b_proj.
 - error budget (sim-validated, deterministic inputs): rel_err 1.324e-2
   vs 2e-2 gate.
"""
import numpy as np
import concourse.bacc as bacc
import concourse.mybir as mybir
from concourse.tile import TileContext

FP32 = mybir.dt.float32
F16 = mybir.dt.float16
F8 = mybir.dt.float8e4
AF = mybir.ActivationFunctionType
ALU = mybir.AluOpType
DR = mybir.MatmulPerfMode.DoubleRow

SX = 16.0                      # x fp8 scale (q,k path)
SW = 64.0                      # W fp8 scale
DESCALE = 1.0 / (SX * SW)      # 2^-10

N = 17
C = 512
H = 8
HD = 64
SCALE = HD ** -0.5
GB = 7
GR = GB * N  # 119

N_CORES = 8
B_FULL = 8192
B_CORE = B_FULL // N_CORES     # 1024
B_PAD = 1036                   # 148 groups of 7
G_CHUNK = 8

# Bonechain, batched into rounds of steps (pp,p,c) with c=p+1 and regular
# offset strides in the flat (n*17+m)*8 element space of a [b,(n,m,h)] row.
# Each entry: (dst0, dst_stride, S, src0, src_stride); dst covers the
# (p,c)&(c,p) pair via a 2x128-element sub-AP.
CHAIN_ROUNDS = [
    (152, 432, 3, 8, 24),      # (0,1,2) (0,4,5) (0,7,8)
    (1592, 432, 2, 1176, 24),  # (8,11,12) (8,14,15)
    (296, 432, 3, 152, 432),   # (1,2,3) (4,5,6) (7,8,9)
    (1736, 432, 2, 1592, 432),  # (11,12,13) (14,15,16)
    (1304, 432, 1, 1160, 432),  # (8,9,10)
]
BM_COLS = N * N * H + 296      # chain APs need slack: max 1736+2*432 = 2600

_CACHE = {}


def _build(nc, B_pad, G_chunk):
    assert B_pad % GB == 0
    n_groups = B_pad // GB
    chunks = []
    g0 = 0
    while g0 < n_groups:
        g = min(G_chunk, n_groups - g0)
        chunks.append((g0, g))
        g0 += g

    R_tot = B_pad * N
    Rpad = n_groups * 128  # groups padded to 128 rows in the col space

    xT_d = nc.dram_tensor("xT", [C, Rpad], F16, kind="ExternalInput")
    # q,k path: fp8 DoubleRow; rows a*128+p with a=(kcp*2+j), contraction
    # channel = kcp*256 + j*128 + p; cols j-slabs for x.
    xq8_d = nc.dram_tensor("xq8", [256, 2 * Rpad], F8, kind="ExternalInput")
    w8_d = nc.dram_tensor("w8", [512, 2 * C], F8, kind="ExternalInput")
    wv_d = nc.dram_tensor("wv", [C, C], F16, kind="ExternalInput")
    wpj_d = nc.dram_tensor("wpj", [C, C], F16, kind="ExternalInput")
    yT_d = nc.dram_tensor("yT", [C, R_tot], F16, kind="ExternalOutput")

    with TileContext(nc) as tc:
        with tc.tile_pool(name="persist", bufs=1) as pp, \
             tc.tile_pool(name="xtp", bufs=2) as xt_p, \
             tc.tile_pool(name="qkT", bufs=1) as qkT_p, \
             tc.tile_pool(name="vp", bufs=3) as v_p, \
             tc.tile_pool(name="sstrip", bufs=1) as ss_p, \
             tc.tile_pool(name="bmaj", bufs=2) as bm_p, \
             tc.tile_pool(name="aop", bufs=2) as ao_p, \
             tc.tile_pool(name="outp", bufs=1) as out_p, \
             tc.tile_pool(name="dram", bufs=2, space="DRAM") as dram_p, \
             tc.tile_pool(name="ps", bufs=8, space="PSUM") as ps_p:

            # persistent weights: w8 first (A2's only dependency), then
            # wv/wpj on gpsimd to keep sync/scalar free for the first loads
            w8t = pp.tile([128, 4 * 2 * C], F8)  # [p, (kcp j m)]
            nc.sync.dma_start(
                out=w8t[:].rearrange("p (a m) -> p a m", a=4),
                in_=w8_d[:].rearrange("(a p) m -> p a m", p=128))
            w16t = pp.tile([128, 4 * C], F16)
            nc.gpsimd.dma_start(
                out=w16t[:].rearrange("p (kt m) -> p kt m", kt=4),
                in_=wv_d[:].rearrange("(kt p) m -> p kt m", p=128))
            wpjt = pp.tile([128, 4 * C], F16)    # [p, (kt m)]
            nc.gpsimd.dma_start(
                out=wpjt[:].rearrange("p (kt m) -> p kt m", kt=4),
                in_=wpj_d[:].rearrange("(kt p) m -> p kt m", p=128))
            w4 = w16t[:].rearrange("p (kt m) -> p kt m", kt=4)
            w8v = w8t[:].rearrange("p (kcp j m) -> p kcp j m", kcp=2, j=2)

            def wpjs(kt, mt):
                return wpjt[:].rearrange("p (kt m) -> p kt m", kt=4)[
                    :, kt, mt * 128:(mt + 1) * 128]

            # persistent AT strips (x3, rotating: finish lags spine by 2)
            # cols (j, g, h, n): block-diag over j; memset once -> zeros persist
            at_strips = []
            for pi in range(3):
                at_s = pp.tile([GR, GB * G_chunk * H * N], F16, name=f"at_s{pi}")
                nc.vector.memset(at_s[:], 0.0)
                at_strips.append(at_s)

            loaded = {}

            def emit_loads(ci):
                g0, G = chunks[ci]
                RC2 = G * 128
                lo = g0 * 128
                xt = xt_p.tile([128, 4 * G_chunk * 128], F16, tag="xt",
                               name="xt")
                x4 = xt[:].rearrange("p (kt r) -> p kt r", kt=4)
                for kt in range(4):
                    nc.sync.dma_start(
                        out=x4[:, kt, :RC2],
                        in_=xT_d[kt * 128:(kt + 1) * 128, lo:lo + RC2])
                xq = xt_p.tile([128, 4 * G_chunk * 128], F8, tag="xq",
                               name="xq")
                xq4 = xq[:].rearrange("p (kcp j r) -> p kcp j r", kcp=2, j=2)
                for kcp in range(2):
                    for j in range(2):
                        nc.scalar.dma_start(
                            out=xq4[:, kcp, j, :RC2],
                            in_=xq8_d[kcp * 128:(kcp + 1) * 128,
                                      j * Rpad + lo:j * Rpad + lo + RC2])
                loaded[ci] = (xt, xq)

            def emit_front(ci):
                g0, G = chunks[ci]
                RC = GR * G
                RC2 = G * 128
                r0 = g0 * GR
                lo = g0 * 128

                xt, xq = loaded.pop(ci)
                x4 = xt[:].rearrange("p (kt r) -> p kt r", kt=4)
                xq4 = xq[:].rearrange("p (kcp j r) -> p kcp j r", kcp=2, j=2)

                # A2: qT,kT channel-major via fp8 DoubleRow (1 term)
                qkT = [qkT_p.tile([128, G_chunk * 128], F16, tag=f"qkT{mt}",
                                  name=f"qkT{mt}") for mt in range(8)]
                n_nt2 = (RC2 + 511) // 512
                for mt in range(8):
                    psqs = [ps_p.tile([128, 512], FP32, tag="ps", name="psq")
                            for _ in range(n_nt2)]
                    for kcp in range(2):
                        for nt in range(n_nt2):
                            c0 = nt * 512
                            cw = min(512, RC2 - c0)
                            nc.tensor.matmul(
                                psqs[nt][:, :cw],
                                w8v[:, kcp, :, mt * 128:(mt + 1) * 128],
                                xq4[:, kcp, :, c0:c0 + cw],
                                start=(kcp == 0), stop=(kcp == 1),
                                perf_mode=DR)
                    for nt in range(n_nt2):
                        c0 = nt * 512
                        cw = min(512, RC2 - c0)
                        dst = qkT[mt][:, c0:c0 + cw]
                        if (mt + nt) % 2 == 0:
                            nc.vector.tensor_scalar_mul(dst, psqs[nt][:, :cw],
                                                        DESCALE)
                        else:
                            nc.scalar.activation(dst, psqs[nt][:, :cw],
                                                 AF.Copy, scale=DESCALE)

                vts = []
                # B: scores f16; sstrip cols (g, m', h) with m'=(j',m) 119-wide
                sstrip = ss_p.tile([GR, G_chunk * GR * H], F16, tag="ss",
                                   name="sstrip")
                s4 = sstrip[:].rearrange("p (g m h) -> p g m h", g=G_chunk, h=H)
                for g in range(G):
                    for par in range(2):
                        pss = ps_p.tile([128, 512], FP32, tag="ps", name="pss")
                        for qi in range(4):
                            h = 2 * qi + par
                            mt = h // 2
                            p0 = (h % 2) * 64
                            qs = qkT[mt][p0:p0 + 64, g * 128:g * 128 + GR]
                            ks = qkT[4 + mt][p0:p0 + 64, g * 128:g * 128 + GR]
                            nc.tensor.matmul(pss[:GR, qi * GR:(qi + 1) * GR],
                                             qs, ks, start=True, stop=True)
                        # src (p, hh4, m') -> dst (p, m', h at par::2)
                        srcq = pss[:GR, :4 * GR].rearrange(
                            "p (hh m) -> p m hh", m=GR)
                        dstq = s4[:, g, :, par * 4:(par + 1) * 4]
                        if (g + par) % 2 == 0:
                            nc.vector.tensor_copy(dstq, srcq)
                        else:
                            nc.scalar.copy(dstq, srcq)

                # A3: v row-major f16 per group (stationary = x slice)
                for g in range(G):
                    vt = v_p.tile([GR, C], F16, tag=f"v{g}", name=f"v{g}")
                    psv = ps_p.tile([128, 512], FP32, tag="ps", name="psv")
                    for kt in range(4):
                        nc.tensor.matmul(
                            psv[:, :],
                            x4[:, kt, g * 128:(g + 1) * 128],
                            w4[:, kt, :],
                            start=(kt == 0), stop=(kt == 3))
                    if g % 2 == 0:
                        nc.vector.tensor_copy(vt[:], psv[:GR, :])
                    else:
                        nc.scalar.copy(vt[:], psv[:GR, :])
                    vts.append(vt)

                # conv1a: ONE fat DMA sstrip -> DRAM (119 descs of 15KB);
                # conv1b extracts the block-diag from the DRAM copy.
                st1 = dram_p.tile([GR, G_chunk * GR * H], F16, tag="st1")
                nc.sync.dma_start(out=st1[:, :], in_=sstrip[:, :])
                return {"vts": vts, "st1": st1, "G": G, "RC": RC,
                        "r0": r0, "ci": ci, "x4": x4}

            def emit_spine(st):
                G, RC, r0 = st["G"], st["RC"], st["r0"]
                BC = GB * G
                st1 = st["st1"]
                at_strip = at_strips[st["ci"] % 3]

                # conv1b: staged1 diag -> b-major bmS [b=(j,g), (n, m, h)]
                bmS = bm_p.tile([GB * G_chunk, BM_COLS], F16, tag="bmS")
                for j in range(GB):
                    src = st1[N * j:N * (j + 1), :].rearrange(
                        "n (g mp h) -> g n mp h", g=G_chunk, mp=GR)[
                        :G, :, N * j:N * (j + 1), :].rearrange(
                        "g n m h -> g n (m h)")
                    dst = bmS[j * G:(j + 1) * G, :N * N * H].rearrange(
                        "b (n mh) -> b n mh", n=N)
                    eng = nc.gpsimd if j % 2 == 0 else nc.sync
                    eng.dma_start(out=dst, in_=src)

                # D: bonechain on gpsimd, batched rounds; each op updates
                # (p,c)&(c,p) pairs for S steps via [b, S, 2, H] APs.
                for (d0, ds, S, s0, ss) in CHAIN_ROUNDS:
                    d2 = bmS[:BC, d0:d0 + S * ds].rearrange(
                        "b (s q) -> b s q", s=S)[:, :, :256].rearrange(
                        "b s (t x) -> b s t x", t=2)[:, :, :, :H]
                    src = bmS[:BC, s0:s0 + S * ss].rearrange(
                        "b (s q) -> b s q", s=S)[:, :, :H] \
                        .unsqueeze(2).broadcast_to([BC, S, 2, H])
                    nc.gpsimd.tensor_tensor(out=d2, in0=d2, in1=src,
                                            op=ALU.add)
                    nc.gpsimd.tensor_scalar_mul(d2, d2, 0.5)

                # softmax: exp contiguous -> bmA [b, (n, m, h)]
                bmA = bm_p.tile([GB * G_chunk, N * N * H], F16, tag="bmA")
                nc.scalar.activation(bmA[:BC], bmS[:BC, :N * N * H],
                                     AF.Exp, scale=SCALE)
                # reduce over m: contiguous fold-tree (17 = 8+8+1), all f16
                a4 = bmA[:BC].rearrange("b (n m h) -> b n m h", n=N, m=N)
                zt = bm_p.tile([GB * G_chunk, N * 15 * H], F16, tag="zt")
                z8 = zt[:BC, :N * 8 * H].rearrange("b (n m h) -> b n m h", n=N, m=8)
                z4_ = zt[:BC, N * 8 * H:N * 12 * H].rearrange(
                    "b (n m h) -> b n m h", n=N, m=4)
                z2 = zt[:BC, N * 12 * H:N * 14 * H].rearrange(
                    "b (n m h) -> b n m h", n=N, m=2)
                z1 = zt[:BC, N * 14 * H:N * 15 * H].rearrange(
                    "b (n m h) -> b n m h", n=N, m=1)
                nc.vector.tensor_tensor(out=z8, in0=a4[:, :, 0:8, :],
                                        in1=a4[:, :, 8:16, :], op=ALU.add)
                nc.vector.tensor_tensor(out=z4_, in0=z8[:, :, 0:4, :],
                                        in1=z8[:, :, 4:8, :], op=ALU.add)
                nc.vector.tensor_tensor(out=z2, in0=z4_[:, :, 0:2, :],
                                        in1=z4_[:, :, 2:4, :], op=ALU.add)
                nc.vector.tensor_tensor(out=z1, in0=z2[:, :, 0:1, :],
                                        in1=z2[:, :, 1:2, :], op=ALU.add)
                zs = bm_p.tile([GB * G_chunk, N * H], FP32, tag="zs")
                z4 = zs[:BC].rearrange("b (n h) -> b n h", n=N)
                nc.vector.tensor_tensor(
                    out=z4, in0=z1[:, :, 0, :], in1=a4[:, :, 16, :],
                    op=ALU.add)
                rec = bm_p.tile([GB * G_chunk, N * H], FP32, tag="rec")
                nc.vector.reciprocal(rec[:BC], zs[:BC])
                # normalize: strided reads, contiguous write bmP [b, (m, h, n)]
                bmP = bm_p.tile([GB * G_chunk, N * H * N], F16, tag="bmP")
                p4 = bmP[:BC].rearrange("b (m h n) -> b m h n", m=N, h=H)
                in0 = bmA[:BC].rearrange("b (n m h) -> b m h n", n=N, m=N)
                in1 = rec[:BC].rearrange("b (n h) -> b h n", n=N) \
                    .unsqueeze(1).broadcast_to([BC, N, H, N])
                nc.vector.tensor_tensor(out=p4, in0=in0, in1=in1, op=ALU.mult)

                # conv2a: ONE fat DMA bmP -> DRAM staged2 [(j,g), (m, h, n)]
                st2 = dram_p.tile([GB * G_chunk, N * H * N], F16, tag="st2")
                nc.sync.dma_start(out=st2[:BC, :], in_=bmP[:BC, :])
                # conv2b: staged2 -> at_strip cols (j, g, h, n); (h,n) 272B runs
                at4 = at_strip[:].rearrange(
                    "p (j g h n) -> p j g h n", j=GB, g=G_chunk, h=H)
                for j in range(GB):
                    src = st2[j * G:(j + 1) * G, :].rearrange(
                        "g (m hn) -> m g hn", m=N)
                    dst = at4[N * j:N * (j + 1), j, :G, :, :].rearrange(
                        "p g h n -> p g (h n)")
                    eng = nc.gpsimd if j % 2 == 0 else nc.sync
                    eng.dma_start(out=dst, in_=src)

            def emit_finish(st):
                vts = st["vts"]
                G, RC, r0 = st["G"], st["RC"], st["r0"]
                at_strip = at_strips[st["ci"] % 3]
                at4 = at_strip[:].rearrange(
                    "p (j g h n) -> p g h j n", j=GB, g=G_chunk, h=H)

                # F: AV -> aoT channel-major f16 [128, 4*RC]
                aoT = ao_p.tile([128, 4 * G_chunk * GR], F16, tag="aoT",
                                name="aoT")

                def aoTk(t):
                    return aoT[:, t * G_chunk * GR:(t * G_chunk + G) * GR]

                for q0 in range(0, G, 4):
                    nq = min(4, G - q0)
                    for t in range(4):
                        psa = ps_p.tile([128, 512], FP32, tag="ps", name="psa")
                        for qi in range(nq):
                            g = q0 + qi
                            for hp in range(2):
                                h = 2 * t + hp
                                hr = 2 * (h % 4) + h // 4
                                nc.tensor.matmul(
                                    psa[64 * hp:64 * (hp + 1),
                                        qi * GR:(qi + 1) * GR],
                                    vts[g][:, hr * HD:(hr + 1) * HD],
                                    at4[:, g, h, :, :],
                                    start=True, stop=True)
                        dst = aoT[:, (t * G_chunk + q0) * GR:
                                  (t * G_chunk + q0 + nq) * GR]
                        if t % 2 == 0:
                            nc.vector.tensor_copy(dst, psa[:, :nq * GR])
                        else:
                            nc.scalar.copy(dst, psa[:, :nq * GR])

                # G: proj channel-major: yT[mt*128:, rows] f16
                outT = out_p.tile([128, 4 * G_chunk * GR], F16, tag="outT",
                                  name="outT")
                n_nt = (RC + 4 * GR - 1) // (4 * GR)
                for mt in range(4):
                    for nt in range(n_nt):
                        c0 = nt * 4 * GR
                        cw = min(4 * GR, RC - c0)
                        psp = ps_p.tile([128, 512], FP32, tag="ps", name="psp")
                        for kt in range(4):
                            nc.tensor.matmul(
                                psp[:, :cw], wpjs(kt, mt),
                                aoTk(kt)[:, c0:c0 + cw],
                                start=(kt == 0), stop=(kt == 3))
                        dst = outT[:, mt * G_chunk * GR + c0:
                                   mt * G_chunk * GR + c0 + cw]
                        if (mt + nt) % 2 == 0:
                            nc.scalar.copy(dst, psp[:, :cw])
                        else:
                            nc.vector.tensor_copy(dst, psp[:, :cw])
                for mt in range(4):
                    nc.scalar.dma_start(
                        out=yT_d[mt * 128:(mt + 1) * 128, r0:r0 + RC],
                        in_=outT[:, mt * G_chunk * GR:mt * G_chunk * GR + RC])

            # software pipeline: loads(i+1); front(i); spine(i); finish(i-2)
            window = []
            emit_loads(0)
            for ci in range(len(chunks)):
                if ci + 1 < len(chunks):
                    emit_loads(ci + 1)
                st = emit_front(ci)
                emit_spine(st)
                window.append(st)
                if len(window) > 2:
                    emit_finish(window.pop(0))
            for st in window:
                emit_finish(st)
    return nc


def _get_nc():
    key = (B_PAD, G_CHUNK)
    if key not in _CACHE:
        nc = bacc.Bacc(
            "TRN2", target_bir_lowering=False, debug=False,
            enable_asserts=False, num_devices=N_CORES,
        )
        _build(nc, B_pad=B_PAD, G_chunk=G_CHUNK)
        nc.compile()
        _CACHE[key] = nc
    return _CACHE[key]


LAST_RESULTS = None


def kernel(x, W_qkv, W_proj, b_proj):
    import os
    global LAST_RESULTS
    from concourse.bass_utils import run_bass_kernel_spmd

    x = np.asarray(x, dtype=np.float32)
    W_qkv = np.asarray(W_qkv, dtype=np.float32)
    W_proj = np.asarray(W_proj, dtype=np.float32)
    b_proj = np.asarray(b_proj, dtype=np.float32)
    B, N_, C_ = x.shape
    assert (B, N_, C_) == (B_FULL, N, C)

    import ml_dtypes
    E4 = ml_dtypes.float8_e4m3
    wv16 = np.ascontiguousarray(W_qkv[:, 2 * C:]).astype(np.float16)
    hperm = np.array([2 * (h % 4) + h // 4 for h in range(H)])
    rowperm = (hperm[:, None] * HD + np.arange(HD)[None, :]).reshape(-1)
    wpj16 = np.ascontiguousarray(W_proj[rowperm]).astype(np.float16)
    # w8 rows a*128+p with a=(kcp*2+j): channel = kcp*256 + j*128 + p
    wq = (W_qkv[:, :2 * C] * SW).reshape(2, 2, 128, 2 * C)
    w8 = np.asarray(wq, dtype=E4).reshape(512, 2 * C)

    nc = _get_nc()
    n_groups = B_PAD // GB
    Rpad = n_groups * 128
    in_maps = []
    for c in range(N_CORES):
        xs = x[c * B_CORE:(c + 1) * B_CORE].reshape(-1, C).astype(np.float16)
        xs = np.concatenate(
            [xs, np.zeros(((B_PAD - B_CORE) * N, C), np.float16)], axis=0)
        xp = np.zeros((n_groups, 128, C), np.float16)
        xp[:, :GR, :] = xs.reshape(n_groups, GR, C)
        xT = np.ascontiguousarray(xp.reshape(Rpad, C).T)  # [C, Rpad]
        # xq8 [256, 2*Rpad]: rows kcp*128+p, cols j*Rpad + r
        x8 = np.asarray(xT.astype(np.float32) * SX, dtype=E4)  # [C, Rpad]
        x8 = x8.reshape(2, 2, 128, Rpad)                 # [kcp, j, p, r]
        xq8 = np.ascontiguousarray(
            x8.transpose(0, 2, 1, 3)).reshape(256, 2 * Rpad)
        in_maps.append({"xT": xT, "xq8": xq8, "w8": w8,
                        "wv": wv16, "wpj": wpj16})
    trace = bool(os.environ.get("KERNEL_TRACE"))
    res = run_bass_kernel_spmd(nc, in_maps, list(range(N_CORES)), trace=trace)
    LAST_RESULTS = res
    outs = []
    for c in range(N_CORES):
        yT = res.results[c]["yT"]                      # [512, R_tot] f16
        yc = yT[:, :B_CORE * N].astype(np.float32).T   # [rows, 512]
        outs.append(yc.reshape(B_CORE, N, C))
    y = np.concatenate(outs, axis=0)
    return y + b_proj[None, None, :]
